# revision 1
# baseline (speedup 1.0000x reference)
"""CoAttention kernel for 8 TRN2 NeuronCores.

Sharding: batch (4) x role (2) = 8 cores, no collectives.
  core 2b   ("query" role):    computes out2[b] (query_att path)
  core 2b+1 ("exemplar" role): computes out1[b] (exemplar_att path)

Both roles run the SAME program on different data, exploiting the symmetry
  out2 = W2a @ gate(softmax_n(A) applied to ex)      + W2b @ input_2
  out1 = W1a @ gate(softmax_m(A^T) applied to q)     + W1b @ input_1
with A(X, Y, Wh) = (Wh @ X)^T @ Y.  Role Q: X=input_1, Y=input_2, Wh=W_e.
Role E: X=input_2, Y=input_1, Wh=W_e^T (then A' = A^T and the "column"
softmax of A' is the row softmax of A).

Per-core program (C=256, n = X pixels, m = Y pixels):
  EC = Wh @ X                    [C, n]
  for each m-chunk (512):
    for each n-chunk (128):
      A_t  = EC_chunk^T @ Y_chunk          (PE, fp32r, PSUM)
      P_t  = exp(A_t - KEXP)               (ACT, PSUM->SBUF)
      cs  += P_t                           (DVE, partial colsum over n)
      U   += X_chunk @ P_t                 (PE, accumulated in PSUM)
    colsum = ones^T @ cs                   (PE)   -> recip (DVE)
    gdot   = gate_w^T @ U                  (PE)
    scale  = sigmoid(gdot*recip)*recip     (ACT/DVE, [1,512])
    bcast  = ones_col @ scale              (PE outer product -> [128,512])
    out    = WaT^T @ (U*bcast) + WbT^T @ Y (PE) -> DMA
"""

import os
import numpy as np

import concourse.bass as bass
import concourse.bacc as bacc
import concourse.tile as tile
from concourse import mybir
from concourse import bass_utils

F32 = mybir.dt.float32
F32R = mybir.dt.float32r

B = 4
C = 256
H = 64
W = 64
HW = H * W
KEXP = 20.0  # constant subtracted before exp (softmax-invariant)

# knobs (module-level so a test harness can tweak before first call)
TRACE = False
MM_DTYPE = F32R  # matmul operand dtype tag (f32r = full-rate fp32 on PE)
DEBUG_TAPS = False  # add DRAM taps of intermediates (sim debugging only)

_COMPILED = {}


def _build_nc(n_pix, m_pix, rep=1):
    nc = bacc.Bacc(
        "TRN2",
        target_bir_lowering=False,
        debug=False,
        enable_asserts=True,
        num_devices=8,
    )
    X = nc.dram_tensor("x", [C, n_pix], F32R, kind="ExternalInput").ap()
    XT = nc.dram_tensor("xt", [n_pix, C], F32R, kind="ExternalInput").ap()
    Y = nc.dram_tensor("y", [C, m_pix], F32R, kind="ExternalInput").ap()
    WHT = nc.dram_tensor("wht", [C, C], F32R, kind="ExternalInput").ap()
    WAT = nc.dram_tensor("wat", [C, C], F32R, kind="ExternalInput").ap()
    WBT = nc.dram_tensor("wbt", [C, C], F32R, kind="ExternalInput").ap()
    GW = nc.dram_tensor("gw", [C, 1], F32R, kind="ExternalInput").ap()
    ONESC = nc.dram_tensor("onescol", [128, 1], F32R, kind="ExternalInput").ap()
    ONESR = nc.dram_tensor("onesrow", [1, 128], F32R, kind="ExternalInput").ap()
    OUT = nc.dram_tensor("out", [C, m_pix], F32, kind="ExternalOutput").ap()
    taps = {}
    if DEBUG_TAPS:
        for nm, shp in [("d_ec", [128, 2, n_pix]), ("d_p0", [128, 512]),
                        ("d_u0", [128, 512]),
                        ("d_u1", [128, 512]), ("d_recip", [1, 512]),
                        ("d_gd", [1, 512]), ("d_scale", [1, 512]),
                        ("d_gated0", [128, 512])]:
            taps[nm] = nc.dram_tensor(nm, shp, F32R, kind="ExternalOutput").ap()

    NCH = n_pix // 128
    MCH = m_pix // 512
    NK = n_pix // 512  # 512-wide n chunks for the EC phase
    Exp = mybir.ActivationFunctionType.Exp
    Copy = mybir.ActivationFunctionType.Copy
    Sigmoid = mybir.ActivationFunctionType.Sigmoid

    def r(ap):
        if MM_DTYPE is None or ap.dtype == MM_DTYPE:
            return ap
        return ap.bitcast(MM_DTYPE)

    with tile.TileContext(nc) as tc:
        with (
            nc.allow_low_precision(reason="fp32r matmul operand rounding"),
            tc.tile_pool(name="persist", bufs=1) as persist,
            tc.tile_pool(name="psA", bufs=3, space=bass.MemorySpace.PSUM) as psA,
            tc.tile_pool(name="psU", bufs=2, space=bass.MemorySpace.PSUM) as psU,
            tc.tile_pool(name="psO", bufs=1, space=bass.MemorySpace.PSUM) as psO,
            tc.tile_pool(name="pwork", bufs=4) as pwork,
            tc.tile_pool(name="accp", bufs=2) as accp,
            tc.tile_pool(name="upool", bufs=2) as upool,
            tc.tile_pool(name="opool", bufs=2) as opool,
            tc.tile_pool(name="small", bufs=2) as small,
        ):
            # ---- persistent loads, ordered+chunked by first consumption ----
            Xr = X.rearrange("(ci p) n -> p ci n", p=128)
            Yr = Y.rearrange("(ci p) m -> p ci m", p=128)
            XTr = XT.rearrange("(a p) c -> p a c", p=128)
            wht_sb = persist.tile([128, 2, C], F32R)
            nc.sync.dma_start(out=wht_sb, in_=WHT.rearrange("(ci p) d -> p ci d", p=128))
            ones_col = persist.tile([128, 1], F32R)
            nc.sync.dma_start(out=ones_col, in_=ONESC)
            ones_row = persist.tile([1, 128], F32R)
            nc.sync.dma_start(out=ones_row, in_=ONESR)
            x_sb = persist.tile([128, 2, n_pix], F32R)
            for nk in range(NK):
                nsl = slice(nk * 512, (nk + 1) * 512)
                for ci in range(2):
                    nc.sync.dma_start(out=x_sb[:, ci, nsl], in_=Xr[:, ci, nsl])
            y_sb = persist.tile([128, 2, m_pix], F32R)
            for ci in range(2):
                nc.sync.dma_start(out=y_sb[:, ci, 0:512], in_=Yr[:, ci, 0:512])
            xT_sb = persist.tile([128, NCH, C], F32R)
            for a in range(0, NCH, 4):
                nc.sync.dma_start(out=xT_sb[:, a:a + 4, :], in_=XTr[:, a:a + 4, :])
            for mk in range(1, MCH):
                msl_ = slice(mk * 512, (mk + 1) * 512)
                for ci in range(2):
                    nc.sync.dma_start(out=y_sb[:, ci, msl_], in_=Yr[:, ci, msl_])
            wat_sb = persist.tile([128, 2, C], F32R)
            nc.sync.dma_start(out=wat_sb, in_=WAT.rearrange("(ci p) o -> p ci o", p=128))
            wbt_sb = persist.tile([128, 2, C], F32R)
            nc.sync.dma_start(out=wbt_sb, in_=WBT.rearrange("(ci p) o -> p ci o", p=128))
            gw_sb = persist.tile([128, 2, 1], F32R)
            nc.sync.dma_start(out=gw_sb, in_=GW.rearrange("(ci p) o -> p ci o", p=128))
            negk128 = persist.tile([128, 1], F32)
            nc.vector.memset(negk128, -KEXP)
            zero1 = persist.tile([1, 1], F32)
            nc.vector.memset(zero1, 0.0)
            ec_sb = persist.tile([128, 2, n_pix], F32R)

            # ---- EC = Wh @ X ----
            for dj in range(2):
                for nk in range(NK):
                    nsl = slice(nk * 512, (nk + 1) * 512)
                    ec_ps = psA.tile([128, 512], F32, tag="a")
                    for ci in range(2):
                        nc.tensor.matmul(
                            ec_ps,
                            r(wht_sb[:, ci, dj * 128:(dj + 1) * 128]),
                            r(x_sb[:, ci, nsl]),
                            start=(ci == 0),
                            stop=(ci == 1),
                        )
                    nc.scalar.activation(ec_sb[:, dj, nsl], ec_ps, Copy)
            if DEBUG_TAPS:
                nc.sync.dma_start(out=taps["d_ec"], in_=ec_sb)

            # ---- main loop over m-chunks (rep>1 = timing-only replay) ----
            for mj in [mj for _ in range(rep) for mj in range(MCH)]:
                msl = slice(mj * 512, (mj + 1) * 512)
                u_ps0 = psU.tile([128, 512], F32, tag="u0")
                u_ps1 = psU.tile([128, 512], F32, tag="u1")
                cs_acc = accp.tile([128, 512], F32R)
                for nj in range(NCH):
                    nsl128 = slice(nj * 128, (nj + 1) * 128)
                    a_ps = psA.tile([128, 512], F32, tag="a")
                    for di in range(2):
                        nc.tensor.matmul(
                            a_ps,
                            r(ec_sb[:, di, nsl128]),
                            r(y_sb[:, di, msl]),
                            start=(di == 0),
                            stop=(di == 1),
                        )
                    p_sb = pwork.tile([128, 512], F32R, tag="p")
                    nc.scalar.activation(p_sb, a_ps, Exp, bias=negk128, scale=1.0)
                    if DEBUG_TAPS and mj == 0 and nj == 0:
                        nc.sync.dma_start(out=taps["d_p0"], in_=p_sb)
                    if nj == 0:
                        nc.vector.tensor_copy(cs_acc, p_sb)
                    else:
                        nc.vector.tensor_add(cs_acc, cs_acc, p_sb)
                    nc.tensor.matmul(
                        u_ps0,
                        r(xT_sb[:, nj, 0:128]),
                        r(p_sb),
                        start=(nj == 0),
                        stop=(nj == NCH - 1),
                    )
                    nc.tensor.matmul(
                        u_ps1,
                        r(xT_sb[:, nj, 128:256]),
                        r(p_sb),
                        start=(nj == 0),
                        stop=(nj == NCH - 1),
                    )

                # partition-reduce colsum on PE, then reciprocal
                cs_ps = psO.tile([1, 512], F32, tag="o")
                nc.tensor.matmul(cs_ps, r(ones_col), r(cs_acc))
                recip_sb = small.tile([1, 512], F32R, tag="recip")
                nc.vector.reciprocal(recip_sb, cs_ps)
                if DEBUG_TAPS and mj == 0:
                    nc.sync.dma_start(out=taps["d_recip"], in_=recip_sb)

                # copy U out of PSUM
                u_sb0 = upool.tile([128, 512], F32R, tag="usb0")
                u_sb1 = upool.tile([128, 512], F32R, tag="usb1")
                nc.scalar.activation(u_sb0, u_ps0, Copy)
                nc.scalar.activation(u_sb1, u_ps1, Copy)
                if DEBUG_TAPS and mj == 0:
                    nc.sync.dma_start(out=taps["d_u0"], in_=u_sb0)
                    nc.sync.dma_start(out=taps["d_u1"], in_=u_sb1)

                # gate dot product
                gd_ps = psO.tile([1, 512], F32, tag="o")
                nc.tensor.matmul(gd_ps, r(gw_sb[:, 0, :]), r(u_sb0), start=True, stop=False)
                nc.tensor.matmul(gd_ps, r(gw_sb[:, 1, :]), r(u_sb1), start=False, stop=True)

                # scale vector: sigmoid(gdot/colsum)/colsum
                t_sb = small.tile([1, 512], F32R, tag="t")
                nc.vector.tensor_mul(t_sb, gd_ps, recip_sb)
                e_sb = small.tile([1, 512], F32, tag="e")
                nc.scalar.activation(e_sb, t_sb, Exp, bias=zero1, scale=-1.0)
                ep1_sb = small.tile([1, 512], F32, tag="ep1")
                nc.vector.tensor_scalar_add(ep1_sb, e_sb, 1.0)
                g_sb = small.tile([1, 512], F32R, tag="g")
                nc.vector.reciprocal(g_sb, ep1_sb)
                scale_sb = small.tile([1, 512], F32R, tag="scale")
                nc.vector.tensor_mul(scale_sb, g_sb, recip_sb)
                if DEBUG_TAPS and mj == 0:
                    nc.sync.dma_start(out=taps["d_gd"], in_=t_sb)
                    nc.sync.dma_start(out=taps["d_scale"], in_=scale_sb)

                # broadcast scale along partitions via outer product
                bc_ps = psO.tile([128, 512], F32, tag="o")
                nc.tensor.matmul(bc_ps, r(ones_row), r(scale_sb))

                gated0 = upool.tile([128, 512], F32R, tag="gated0")
                gated1 = upool.tile([128, 512], F32R, tag="gated1")
                nc.vector.tensor_mul(gated0, u_sb0, bc_ps)
                nc.vector.tensor_mul(gated1, u_sb1, bc_ps)
                if DEBUG_TAPS and mj == 0:
                    nc.sync.dma_start(out=taps["d_gated0"], in_=gated0)

                # final 1x1 conv: out = WaT^T @ gated + WbT^T @ Y
                for oj in range(2):
                    osl = slice(oj * 128, (oj + 1) * 128)
                    o_ps = psO.tile([128, 512], F32, tag="o")
                    gated = [gated0, gated1]
                    for ci in range(2):
                        nc.tensor.matmul(
                            o_ps,
                            r(wat_sb[:, ci, osl]),
                            r(gated[ci]),
                            start=(ci == 0),
                            stop=False,
                        )
                    for ci in range(2):
                        nc.tensor.matmul(
                            o_ps,
                            r(wbt_sb[:, ci, osl]),
                            r(y_sb[:, ci, msl]),
                            start=False,
                            stop=(ci == 1),
                        )
                    o_sb = opool.tile([128, 512], F32, tag="osb")
                    nc.scalar.activation(o_sb, o_ps, Copy)
                    nc.sync.dma_start(out=OUT[osl, msl], in_=o_sb)

    nc.compile()
    return nc


def _get_compiled(n_pix, m_pix, rep=1):
    key = (n_pix, m_pix, rep, str(MM_DTYPE))
    if key not in _COMPILED:
        _COMPILED[key] = _build_nc(n_pix, m_pix, rep)
    return _COMPILED[key]


def _in_maps(input_1, input_2, W_e, gate_w, W1, W2):
    ex = np.ascontiguousarray(input_1.reshape(B, C, HW), dtype=np.float32)
    q = np.ascontiguousarray(input_2.reshape(B, C, HW), dtype=np.float32)
    W_e = np.asarray(W_e, dtype=np.float32)
    gate_w = np.asarray(gate_w, dtype=np.float32).reshape(C, 1)
    W1 = np.asarray(W1, dtype=np.float32)
    W2 = np.asarray(W2, dtype=np.float32)

    c = np.ascontiguousarray
    onescol = np.ones((128, 1), np.float32)
    onesrow = np.ones((1, 128), np.float32)
    maps = []
    for b in range(B):
        # role Q -> out2[b]
        maps.append({
            "x": ex[b], "xt": c(ex[b].T), "y": q[b],
            "wht": c(W_e.T), "wat": c(W2[:, :C].T), "wbt": c(W2[:, C:].T),
            "gw": gate_w, "onescol": onescol, "onesrow": onesrow,
        })
        # role E -> out1[b]
        maps.append({
            "x": q[b], "xt": c(q[b].T), "y": ex[b],
            "wht": c(W_e), "wat": c(W1[:, :C].T), "wbt": c(W1[:, C:].T),
            "gw": gate_w, "onescol": onescol, "onesrow": onesrow,
        })
    return maps


def kernel(input_1, input_2, W_e, gate_w, W1, W2):
    nc = _get_compiled(HW, HW)
    maps = _in_maps(input_1, input_2, W_e, gate_w, W1, W2)
    res = bass_utils.run_bass_kernel_spmd(
        nc, maps, core_ids=list(range(8)), trace=TRACE
    )
    kernel.last_results = res
    out1 = np.stack([res.results[2 * b + 1]["out"] for b in range(B)])
    out2 = np.stack([res.results[2 * b]["out"] for b in range(B)])
    return out1.reshape(B, C, H, W), out2.reshape(B, C, H, W)



# revision 2
# speedup vs baseline: 1.3724x; 1.3724x over previous
"""CoAttention kernel v2 for 8 TRN2 NeuronCores.

Sharding: batch (4) x role (2) = 8 cores, no collectives (see the role
symmetry note in the docstring of the original kernel).

v2 changes vs baseline:
  1. A and U matmuls use bf16 operands (PSUM accumulation stays fp32).
     fp32r self-loading matmuls pay a serialized ~107ns 4-byte
     LDWEIGHTS per instruction; bf16 weights get fast-weight-load.
     Micro-measured per-MM: f32r 332ns -> bf16 293ns.
  2. Software pipelining: the per-nj chain A->Exp->U is serialized on
     the in-order PE queue in the baseline (the U matmuls' wait for the
     ACT Exp stalls the queue, exposing ~700ns of ACT latency per nj).
     v2 issues U(nj-2) after A(nj), so Exp(nj-2) has ~2 A-pair times to
     complete before U(nj-2) reaches the head of the PE queue.
  3. The per-mj tail (colsum/gate/scale/bcast/out-conv) is a long
     cross-engine dependency chain; v2 interleaves the previous mj's
     tail groups into the next mj's inner loop at spaced slots so their
     waits are satisfied on arrival.
  EC stays fp32r (moving x / stationary Wh) for precision; ec output is
  rounded to bf16 once. Expected rel-err ~5e-3 (vs 5.5e-4 all-f32r).

Per-core program (C=256, n = X pixels, m = Y pixels):
  EC = Wh @ X                    [C, n]   (f32r, bf16 out)
  for each m-chunk (512):
    for each n-chunk (128):
      A_t  = EC_chunk^T @ Y_chunk          (PE bf16, fp32 PSUM)
      P_t  = exp(A_t - KEXP)               (ACT, PSUM->SBUF, bf16)
      cs  += P_t                           (DVE, f32 acc += bf16)
      U   += X_chunk @ P_t                 (PE bf16, fp32 PSUM)
    colsum = ones^T @ cs                   (PE f32r)  -> recip (DVE)
    gdot   = gate_w^T @ U                  (PE bf16)
    scale  = sigmoid(gdot*recip)*recip     (ACT/DVE, [1,512])
    bcast  = ones_col @ scale              (PE f32r outer product)
    out    = WaT^T @ (U*bcast) + WbT^T @ Y (PE bf16) -> DMA
"""

import numpy as np
import ml_dtypes

import concourse.bass as bass
import concourse.bacc as bacc
import concourse.tile as tile
from concourse import mybir
from concourse import bass_utils

F32 = mybir.dt.float32
F32R = mybir.dt.float32r
BF16 = mybir.dt.bfloat16

B = 4
C = 256
H = 64
W = 64
HW = H * W
KEXP = 20.0  # constant subtracted before exp (softmax-invariant)

TRACE = False
AHEAD = 2  # U(nj-AHEAD) issued after A(nj)

_COMPILED = {}


def _build_nc(n_pix, m_pix, rep=1):
    nc = bacc.Bacc(
        "TRN2",
        target_bir_lowering=False,
        debug=False,
        enable_asserts=True,
        num_devices=8,
    )
    X = nc.dram_tensor("x", [C, n_pix], F32R, kind="ExternalInput").ap()
    XT = nc.dram_tensor("xt", [n_pix, C], BF16, kind="ExternalInput").ap()
    Y = nc.dram_tensor("y", [C, m_pix], BF16, kind="ExternalInput").ap()
    WHT = nc.dram_tensor("wht", [C, C], F32R, kind="ExternalInput").ap()
    WAT = nc.dram_tensor("wat", [C, C], BF16, kind="ExternalInput").ap()
    WBT = nc.dram_tensor("wbt", [C, C], BF16, kind="ExternalInput").ap()
    GW = nc.dram_tensor("gw", [C, 1], BF16, kind="ExternalInput").ap()
    ONESC = nc.dram_tensor("onescol", [128, 1], F32R, kind="ExternalInput").ap()
    ONESR = nc.dram_tensor("onesrow", [1, 128], F32R, kind="ExternalInput").ap()
    OUT = nc.dram_tensor("out", [C, m_pix], F32, kind="ExternalOutput").ap()

    NCH = n_pix // 128
    MCH = m_pix // 512
    NK = n_pix // 512  # 512-wide n chunks for the EC phase
    Exp = mybir.ActivationFunctionType.Exp
    Copy = mybir.ActivationFunctionType.Copy

    with tile.TileContext(nc) as tc:
        with (
            nc.allow_low_precision(reason="bf16 matmul operands"),
            tc.tile_pool(name="persist", bufs=1) as persist,
            tc.tile_pool(name="psA", bufs=3, space=bass.MemorySpace.PSUM) as psA,
            tc.tile_pool(name="psU", bufs=2, space=bass.MemorySpace.PSUM) as psU,
            tc.tile_pool(name="psO", bufs=1, space=bass.MemorySpace.PSUM) as psO,
            tc.tile_pool(name="pwork", bufs=6) as pwork,
            tc.tile_pool(name="accp", bufs=2) as accp,
            tc.tile_pool(name="upool", bufs=2) as upool,
            tc.tile_pool(name="opool", bufs=2) as opool,
            tc.tile_pool(name="small", bufs=2) as small,
        ):
            # ---- persistent loads, ordered+chunked by first consumption ----
            Xr = X.rearrange("(ci p) n -> p ci n", p=128)
            Yr = Y.rearrange("(ci p) m -> p ci m", p=128)
            XTr = XT.rearrange("(a p) c -> p a c", p=128)
            wht_sb = persist.tile([128, 2, C], F32R)
            nc.sync.dma_start(out=wht_sb, in_=WHT.rearrange("(ci p) d -> p ci d", p=128))
            ones_col = persist.tile([128, 1], F32R)
            nc.sync.dma_start(out=ones_col, in_=ONESC)
            ones_row = persist.tile([1, 128], F32R)
            nc.sync.dma_start(out=ones_row, in_=ONESR)
            x_sb = persist.tile([128, 2, n_pix], F32R)
            for nk in range(NK):
                nsl = slice(nk * 512, (nk + 1) * 512)
                for ci in range(2):
                    nc.sync.dma_start(out=x_sb[:, ci, nsl], in_=Xr[:, ci, nsl])
            y_sb = persist.tile([128, 2, m_pix], BF16)
            for ci in range(2):
                nc.sync.dma_start(out=y_sb[:, ci, 0:512], in_=Yr[:, ci, 0:512])
            xT_sb = persist.tile([128, NCH, C], BF16)
            for a in range(0, NCH, 4):
                nc.sync.dma_start(out=xT_sb[:, a:a + 4, :], in_=XTr[:, a:a + 4, :])
            for mk in range(1, MCH):
                msl_ = slice(mk * 512, (mk + 1) * 512)
                for ci in range(2):
                    nc.sync.dma_start(out=y_sb[:, ci, msl_], in_=Yr[:, ci, msl_])
            wat_sb = persist.tile([128, 2, C], BF16)
            nc.sync.dma_start(out=wat_sb, in_=WAT.rearrange("(ci p) o -> p ci o", p=128))
            wbt_sb = persist.tile([128, 2, C], BF16)
            nc.sync.dma_start(out=wbt_sb, in_=WBT.rearrange("(ci p) o -> p ci o", p=128))
            gw_sb = persist.tile([128, 2, 1], BF16)
            nc.sync.dma_start(out=gw_sb, in_=GW.rearrange("(ci p) o -> p ci o", p=128))
            negk128 = persist.tile([128, 1], F32)
            nc.vector.memset(negk128, -KEXP)
            zero1 = persist.tile([1, 1], F32)
            nc.vector.memset(zero1, 0.0)
            ec_sb = persist.tile([128, 2, n_pix], BF16)

            # ---- EC = Wh @ X (f32r operands, bf16 result) ----
            for dj in range(2):
                for nk in range(NK):
                    nsl = slice(nk * 512, (nk + 1) * 512)
                    ec_ps = psA.tile([128, 512], F32, tag="a")
                    for ci in range(2):
                        nc.tensor.matmul(
                            ec_ps,
                            wht_sb[:, ci, dj * 128:(dj + 1) * 128],
                            x_sb[:, ci, nsl],
                            start=(ci == 0),
                            stop=(ci == 1),
                        )
                    nc.scalar.activation(ec_sb[:, dj, nsl], ec_ps, Copy)

            # ---- main loop over m-chunks (rep>1 = timing-only replay) ----
            # Tail groups of iteration t are interleaved into iteration
            # t+1's inner loop at these nj slots:
            TAIL_SLOTS = {3: 0, 6: 1, 10: 2, 13: 3, 16: 4}

            def make_tail(msl, u_ps0, u_ps1, cs_acc):
                st = {}

                def g0():  # colsum -> recip; copy U out of PSUM (bf16)
                    cs_ps = psO.tile([1, 512], F32, tag="o")
                    nc.tensor.matmul(cs_ps, ones_col, cs_acc)
                    st["recip"] = small.tile([1, 512], F32R, tag="recip", name="recip")
                    nc.vector.reciprocal(st["recip"], cs_ps)
                    st["u_sb0"] = upool.tile([128, 512], BF16, tag="usb0", name="usb0")
                    st["u_sb1"] = upool.tile([128, 512], BF16, tag="usb1", name="usb1")
                    nc.scalar.activation(st["u_sb0"], u_ps0, Copy)
                    nc.scalar.activation(st["u_sb1"], u_ps1, Copy)

                def g1():  # gate dot product
                    st["gd_ps"] = psO.tile([1, 512], F32, tag="o", name="gdps")
                    nc.tensor.matmul(st["gd_ps"], gw_sb[:, 0, :], st["u_sb0"],
                                     start=True, stop=False)
                    nc.tensor.matmul(st["gd_ps"], gw_sb[:, 1, :], st["u_sb1"],
                                     start=False, stop=True)

                def g2():  # scale = sigmoid(gdot/colsum)/colsum; bcast; gated
                    t_sb = small.tile([1, 512], F32R, tag="t")
                    nc.vector.tensor_mul(t_sb, st["gd_ps"], st["recip"])
                    e_sb = small.tile([1, 512], F32, tag="e")
                    nc.scalar.activation(e_sb, t_sb, Exp, bias=zero1, scale=-1.0)
                    ep1_sb = small.tile([1, 512], F32, tag="ep1")
                    nc.vector.tensor_scalar_add(ep1_sb, e_sb, 1.0)
                    g_sb = small.tile([1, 512], F32R, tag="g")
                    nc.vector.reciprocal(g_sb, ep1_sb)
                    scale_sb = small.tile([1, 512], F32R, tag="scale")
                    nc.vector.tensor_mul(scale_sb, g_sb, st["recip"])
                    bc_ps = psO.tile([128, 512], F32, tag="o")
                    nc.tensor.matmul(bc_ps, ones_row, scale_sb)
                    st["gated0"] = upool.tile([128, 512], BF16, tag="gated0", name="gated0")
                    st["gated1"] = upool.tile([128, 512], BF16, tag="gated1", name="gated1")
                    nc.vector.tensor_mul(st["gated0"], st["u_sb0"], bc_ps)
                    nc.vector.tensor_mul(st["gated1"], st["u_sb1"], bc_ps)

                def out_conv(oj):
                    osl = slice(oj * 128, (oj + 1) * 128)
                    o_ps = psO.tile([128, 512], F32, tag="o")
                    gated = [st["gated0"], st["gated1"]]
                    for ci in range(2):
                        nc.tensor.matmul(o_ps, wat_sb[:, ci, osl], gated[ci],
                                         start=(ci == 0), stop=False)
                    for ci in range(2):
                        nc.tensor.matmul(o_ps, wbt_sb[:, ci, osl], y_sb[:, ci, msl],
                                         start=False, stop=(ci == 1))
                    o_sb = opool.tile([128, 512], F32, tag="osb")
                    nc.scalar.activation(o_sb, o_ps, Copy)
                    nc.sync.dma_start(out=OUT[osl, msl], in_=o_sb)

                return [g0, g1, g2, lambda: out_conv(0), lambda: out_conv(1)]

            pending = None
            for mj in [mj for _ in range(rep) for mj in range(MCH)]:
                msl = slice(mj * 512, (mj + 1) * 512)
                u_ps0 = psU.tile([128, 512], F32, tag="u0")
                u_ps1 = psU.tile([128, 512], F32, tag="u1")
                cs_acc = accp.tile([128, 512], F32R, tag="cs")
                p_tiles = {}

                def emit_U(nj):
                    nc.tensor.matmul(u_ps0, xT_sb[:, nj, 0:128], p_tiles[nj],
                                     start=(nj == 0), stop=(nj == NCH - 1))
                    nc.tensor.matmul(u_ps1, xT_sb[:, nj, 128:256], p_tiles[nj],
                                     start=(nj == 0), stop=(nj == NCH - 1))
                    del p_tiles[nj]

                for nj in range(NCH):
                    nsl128 = slice(nj * 128, (nj + 1) * 128)
                    a_ps = psA.tile([128, 512], F32, tag="a")
                    for di in range(2):
                        nc.tensor.matmul(
                            a_ps,
                            ec_sb[:, di, nsl128],
                            y_sb[:, di, msl],
                            start=(di == 0),
                            stop=(di == 1),
                        )
                    p_sb = pwork.tile([128, 512], BF16, tag="p")
                    p_tiles[nj] = p_sb
                    nc.scalar.activation(p_sb, a_ps, Exp, bias=negk128, scale=1.0)
                    if nj == 0:
                        nc.vector.tensor_copy(cs_acc, p_sb)
                    else:
                        nc.vector.tensor_add(cs_acc, cs_acc, p_sb)
                    if nj >= AHEAD:
                        emit_U(nj - AHEAD)
                    if pending is not None and nj in TAIL_SLOTS:
                        pending[TAIL_SLOTS[nj]]()
                for nj in range(NCH - AHEAD, NCH):
                    emit_U(nj)
                pending = make_tail(msl, u_ps0, u_ps1, cs_acc)
            for g in pending:
                g()

    nc.compile()
    return nc


def _get_compiled(n_pix, m_pix, rep=1):
    key = (n_pix, m_pix, rep)
    if key not in _COMPILED:
        _COMPILED[key] = _build_nc(n_pix, m_pix, rep)
    return _COMPILED[key]


def _in_maps(input_1, input_2, W_e, gate_w, W1, W2):
    ex = np.ascontiguousarray(input_1.reshape(B, C, HW), dtype=np.float32)
    q = np.ascontiguousarray(input_2.reshape(B, C, HW), dtype=np.float32)
    W_e = np.asarray(W_e, dtype=np.float32)
    gate_w = np.asarray(gate_w, dtype=np.float32).reshape(C, 1)
    W1 = np.asarray(W1, dtype=np.float32)
    W2 = np.asarray(W2, dtype=np.float32)

    bf = ml_dtypes.bfloat16

    def cb(a):  # contiguous bf16
        return np.ascontiguousarray(np.asarray(a).astype(bf))

    onescol = np.ones((128, 1), np.float32)
    onesrow = np.ones((1, 128), np.float32)
    gw_bf = np.ascontiguousarray(gate_w.astype(bf))
    maps = []
    for b in range(B):
        # role Q -> out2[b]
        maps.append({
            "x": ex[b], "xt": cb(ex[b].T), "y": cb(q[b]),
            "wht": np.ascontiguousarray(W_e.T),
            "wat": cb(W2[:, :C].T), "wbt": cb(W2[:, C:].T),
            "gw": gw_bf, "onescol": onescol, "onesrow": onesrow,
        })
        # role E -> out1[b]
        maps.append({
            "x": q[b], "xt": cb(q[b].T), "y": cb(ex[b]),
            "wht": np.ascontiguousarray(W_e),
            "wat": cb(W1[:, :C].T), "wbt": cb(W1[:, C:].T),
            "gw": gw_bf, "onescol": onescol, "onesrow": onesrow,
        })
    return maps


def kernel(input_1, input_2, W_e, gate_w, W1, W2):
    nc = _get_compiled(HW, HW)
    maps = _in_maps(input_1, input_2, W_e, gate_w, W1, W2)
    res = bass_utils.run_bass_kernel_spmd(
        nc, maps, core_ids=list(range(8)), trace=TRACE
    )
    kernel.last_results = res
    out1 = np.stack([res.results[2 * b + 1]["out"] for b in range(B)])
    out2 = np.stack([res.results[2 * b]["out"] for b in range(B)])
    return out1.reshape(B, C, H, W), out2.reshape(B, C, H, W)


# revision 3
# speedup vs baseline: 1.3840x; 1.0085x over previous
"""CoAttention kernel v2 for 8 TRN2 NeuronCores.

Sharding: batch (4) x role (2) = 8 cores, no collectives (see the role
symmetry note in the docstring of the original kernel).

v2 changes vs baseline:
  1. A and U matmuls use bf16 operands (PSUM accumulation stays fp32).
     fp32r self-loading matmuls pay a serialized ~107ns 4-byte
     LDWEIGHTS per instruction; bf16 weights get fast-weight-load.
     Micro-measured per-MM: f32r 332ns -> bf16 293ns.
  2. Software pipelining: the per-nj chain A->Exp->U is serialized on
     the in-order PE queue in the baseline (the U matmuls' wait for the
     ACT Exp stalls the queue, exposing ~700ns of ACT latency per nj).
     v2 issues U(nj-2) after A(nj), so Exp(nj-2) has ~2 A-pair times to
     complete before U(nj-2) reaches the head of the PE queue.
  3. The per-mj tail (colsum/gate/scale/bcast/out-conv) is a long
     cross-engine dependency chain; v2 interleaves the previous mj's
     tail groups into the next mj's inner loop at spaced slots so their
     waits are satisfied on arrival.
  EC stays fp32r (moving x / stationary Wh) for precision; ec output is
  rounded to bf16 once. Expected rel-err ~5e-3 (vs 5.5e-4 all-f32r).

Per-core program (C=256, n = X pixels, m = Y pixels):
  EC = Wh @ X                    [C, n]   (f32r, bf16 out)
  for each m-chunk (512):
    for each n-chunk (128):
      A_t  = EC_chunk^T @ Y_chunk          (PE bf16, fp32 PSUM)
      P_t  = exp(A_t - KEXP)               (ACT, PSUM->SBUF, bf16)
      cs  += P_t                           (DVE, f32 acc += bf16)
      U   += X_chunk @ P_t                 (PE bf16, fp32 PSUM)
    colsum = ones^T @ cs                   (PE f32r)  -> recip (DVE)
    gdot   = gate_w^T @ U                  (PE bf16)
    scale  = sigmoid(gdot*recip)*recip     (ACT/DVE, [1,512])
    bcast  = ones_col @ scale              (PE f32r outer product)
    out    = WaT^T @ (U*bcast) + WbT^T @ Y (PE bf16) -> DMA
"""

import numpy as np
import ml_dtypes

import concourse.bass as bass
import concourse.bacc as bacc
import concourse.tile as tile
from concourse import mybir
from concourse import bass_utils

F32 = mybir.dt.float32
F32R = mybir.dt.float32r
BF16 = mybir.dt.bfloat16

B = 4
C = 256
H = 64
W = 64
HW = H * W
KEXP = 20.0  # constant subtracted before exp (softmax-invariant)

TRACE = False
AHEAD = 2  # U(nj-AHEAD) issued after A(nj)

_COMPILED = {}


def _build_nc(n_pix, m_pix, rep=1):
    nc = bacc.Bacc(
        "TRN2",
        target_bir_lowering=False,
        debug=False,
        enable_asserts=True,
        num_devices=8,
    )
    X = nc.dram_tensor("x", [C, n_pix], F32R, kind="ExternalInput").ap()
    XT = nc.dram_tensor("xt", [n_pix, C], BF16, kind="ExternalInput").ap()
    Y = nc.dram_tensor("y", [C, m_pix], BF16, kind="ExternalInput").ap()
    WHT = nc.dram_tensor("wht", [C, C], F32R, kind="ExternalInput").ap()
    WAT = nc.dram_tensor("wat", [C, C], BF16, kind="ExternalInput").ap()
    WBT = nc.dram_tensor("wbt", [C, C], BF16, kind="ExternalInput").ap()
    GW = nc.dram_tensor("gw", [C, 1], BF16, kind="ExternalInput").ap()
    ONESC = nc.dram_tensor("onescol", [128, 1], F32R, kind="ExternalInput").ap()
    ONESR = nc.dram_tensor("onesrow", [1, 128], F32R, kind="ExternalInput").ap()
    OUT = nc.dram_tensor("out", [C, m_pix], F32, kind="ExternalOutput").ap()

    NCH = n_pix // 128
    MCH = m_pix // 512
    NK = n_pix // 512  # 512-wide n chunks for the EC phase
    Exp = mybir.ActivationFunctionType.Exp
    Copy = mybir.ActivationFunctionType.Copy

    with tile.TileContext(nc) as tc:
        with (
            nc.allow_low_precision(reason="bf16 matmul operands"),
            tc.tile_pool(name="persist", bufs=1) as persist,
            tc.tile_pool(name="psA", bufs=3, space=bass.MemorySpace.PSUM) as psA,
            tc.tile_pool(name="psU", bufs=2, space=bass.MemorySpace.PSUM) as psU,
            tc.tile_pool(name="psO", bufs=1, space=bass.MemorySpace.PSUM) as psO,
            tc.tile_pool(name="pwork", bufs=6) as pwork,
            tc.tile_pool(name="accp", bufs=2) as accp,
            tc.tile_pool(name="upool", bufs=2) as upool,
            tc.tile_pool(name="opool", bufs=2) as opool,
            tc.tile_pool(name="small", bufs=2) as small,
        ):
            # ---- persistent loads, ordered+chunked by first consumption ----
            Xr = X.rearrange("(ci p) n -> p ci n", p=128)
            Yr = Y.rearrange("(ci p) m -> p ci m", p=128)
            XTr = XT.rearrange("(a p) c -> p a c", p=128)
            wht_sb = persist.tile([128, 2, C], F32R)
            nc.sync.dma_start(out=wht_sb, in_=WHT.rearrange("(ci p) d -> p ci d", p=128))
            ones_col = persist.tile([128, 1], F32R)
            nc.sync.dma_start(out=ones_col, in_=ONESC)
            ones_row = persist.tile([1, 128], F32R)
            nc.sync.dma_start(out=ones_row, in_=ONESR)
            x_sb = persist.tile([128, 2, n_pix], F32R)
            for nk in range(NK):
                nsl = slice(nk * 512, (nk + 1) * 512)
                for ci in range(2):
                    nc.sync.dma_start(out=x_sb[:, ci, nsl], in_=Xr[:, ci, nsl])
            y_sb = persist.tile([128, 2, m_pix], BF16)
            for ci in range(2):
                nc.sync.dma_start(out=y_sb[:, ci, 0:512], in_=Yr[:, ci, 0:512])
            xT_sb = persist.tile([128, NCH, C], BF16)
            for a in range(0, NCH, 4):
                nc.sync.dma_start(out=xT_sb[:, a:a + 4, :], in_=XTr[:, a:a + 4, :])
            for mk in range(1, MCH):
                msl_ = slice(mk * 512, (mk + 1) * 512)
                for ci in range(2):
                    nc.sync.dma_start(out=y_sb[:, ci, msl_], in_=Yr[:, ci, msl_])
            wat_sb = persist.tile([128, 2, C], BF16)
            nc.sync.dma_start(out=wat_sb, in_=WAT.rearrange("(ci p) o -> p ci o", p=128))
            wbt_sb = persist.tile([128, 2, C], BF16)
            nc.sync.dma_start(out=wbt_sb, in_=WBT.rearrange("(ci p) o -> p ci o", p=128))
            gw_sb = persist.tile([128, 2, 1], BF16)
            nc.sync.dma_start(out=gw_sb, in_=GW.rearrange("(ci p) o -> p ci o", p=128))
            negk128 = persist.tile([128, 1], F32)
            nc.vector.memset(negk128, -KEXP)
            zero1 = persist.tile([1, 1], F32)
            nc.vector.memset(zero1, 0.0)
            ec_sb = persist.tile([128, 2, n_pix], BF16)

            # ---- EC = Wh @ X (f32r operands, bf16 result) ----
            for dj in range(2):
                for nk in range(NK):
                    nsl = slice(nk * 512, (nk + 1) * 512)
                    ec_ps = psA.tile([128, 512], F32, tag="a")
                    for ci in range(2):
                        nc.tensor.matmul(
                            ec_ps,
                            wht_sb[:, ci, dj * 128:(dj + 1) * 128],
                            x_sb[:, ci, nsl],
                            start=(ci == 0),
                            stop=(ci == 1),
                        )
                    nc.scalar.activation(ec_sb[:, dj, nsl], ec_ps, Copy)

            # ---- main loop over m-chunks (rep>1 = timing-only replay) ----
            # Tail groups of iteration t are interleaved into iteration
            # t+1's inner loop at these nj slots:
            TAIL_SLOTS = {3: 0, 6: 1, 10: 2, 13: 3, 16: 4}

            def make_tail(msl, u_ps0, u_ps1, cs_acc):
                st = {}

                def g0():  # colsum -> recip; copy U out of PSUM (bf16)
                    cs_ps = psO.tile([1, 512], F32, tag="o")
                    nc.tensor.matmul(cs_ps, ones_col, cs_acc)
                    st["recip"] = small.tile([1, 512], F32R, tag="recip", name="recip")
                    nc.vector.reciprocal(st["recip"], cs_ps)
                    st["u_sb0"] = upool.tile([128, 512], BF16, tag="usb0", name="usb0")
                    st["u_sb1"] = upool.tile([128, 512], BF16, tag="usb1", name="usb1")
                    nc.vector.tensor_copy(st["u_sb0"], u_ps0)
                    nc.vector.tensor_copy(st["u_sb1"], u_ps1)

                def g1():  # gate dot product
                    st["gd_ps"] = psO.tile([1, 512], F32, tag="o", name="gdps")
                    nc.tensor.matmul(st["gd_ps"], gw_sb[:, 0, :], st["u_sb0"],
                                     start=True, stop=False)
                    nc.tensor.matmul(st["gd_ps"], gw_sb[:, 1, :], st["u_sb1"],
                                     start=False, stop=True)

                def g2():  # scale = sigmoid(gdot/colsum)/colsum; bcast; gated
                    t_sb = small.tile([1, 512], F32R, tag="t")
                    nc.vector.tensor_mul(t_sb, st["gd_ps"], st["recip"])
                    e_sb = small.tile([1, 512], F32, tag="e")
                    nc.scalar.activation(e_sb, t_sb, Exp, bias=zero1, scale=-1.0)
                    ep1_sb = small.tile([1, 512], F32, tag="ep1")
                    nc.vector.tensor_scalar_add(ep1_sb, e_sb, 1.0)
                    g_sb = small.tile([1, 512], F32R, tag="g")
                    nc.vector.reciprocal(g_sb, ep1_sb)
                    scale_sb = small.tile([1, 512], F32R, tag="scale")
                    nc.vector.tensor_mul(scale_sb, g_sb, st["recip"])
                    bc_ps = psO.tile([128, 512], F32, tag="o")
                    nc.tensor.matmul(bc_ps, ones_row, scale_sb)
                    st["gated0"] = upool.tile([128, 512], BF16, tag="gated0", name="gated0")
                    st["gated1"] = upool.tile([128, 512], BF16, tag="gated1", name="gated1")
                    nc.vector.tensor_mul(st["gated0"], st["u_sb0"], bc_ps)
                    nc.vector.tensor_mul(st["gated1"], st["u_sb1"], bc_ps)

                def out_conv(oj):
                    osl = slice(oj * 128, (oj + 1) * 128)
                    o_ps = psO.tile([128, 512], F32, tag="o")
                    gated = [st["gated0"], st["gated1"]]
                    for ci in range(2):
                        nc.tensor.matmul(o_ps, wat_sb[:, ci, osl], gated[ci],
                                         start=(ci == 0), stop=False)
                    for ci in range(2):
                        nc.tensor.matmul(o_ps, wbt_sb[:, ci, osl], y_sb[:, ci, msl],
                                         start=False, stop=(ci == 1))
                    o_sb = opool.tile([128, 512], F32, tag="osb")
                    nc.vector.tensor_copy(o_sb, o_ps)
                    nc.sync.dma_start(out=OUT[osl, msl], in_=o_sb)

                return [g0, g1, g2, lambda: out_conv(0), lambda: out_conv(1)]

            pending = None
            for mj in [mj for _ in range(rep) for mj in range(MCH)]:
                msl = slice(mj * 512, (mj + 1) * 512)
                u_ps0 = psU.tile([128, 512], F32, tag="u0")
                u_ps1 = psU.tile([128, 512], F32, tag="u1")
                cs_acc = accp.tile([128, 512], F32R, tag="cs")
                p_tiles = {}

                def emit_U(nj):
                    nc.tensor.matmul(u_ps0, xT_sb[:, nj, 0:128], p_tiles[nj],
                                     start=(nj == 0), stop=(nj == NCH - 1))
                    nc.tensor.matmul(u_ps1, xT_sb[:, nj, 128:256], p_tiles[nj],
                                     start=(nj == 0), stop=(nj == NCH - 1))
                    del p_tiles[nj]

                for nj in range(NCH):
                    nsl128 = slice(nj * 128, (nj + 1) * 128)
                    a_ps = psA.tile([128, 512], F32, tag="a")
                    for di in range(2):
                        nc.tensor.matmul(
                            a_ps,
                            ec_sb[:, di, nsl128],
                            y_sb[:, di, msl],
                            start=(di == 0),
                            stop=(di == 1),
                        )
                    p_sb = pwork.tile([128, 512], BF16, tag="p")
                    p_tiles[nj] = p_sb
                    nc.scalar.activation(p_sb, a_ps, Exp, bias=negk128, scale=1.0)
                    if nj == 0:
                        nc.vector.tensor_copy(cs_acc, p_sb)
                    else:
                        nc.vector.tensor_add(cs_acc, cs_acc, p_sb)
                    if nj >= AHEAD:
                        emit_U(nj - AHEAD)
                    if pending is not None and nj in TAIL_SLOTS:
                        pending[TAIL_SLOTS[nj]]()
                for nj in range(NCH - AHEAD, NCH):
                    emit_U(nj)
                pending = make_tail(msl, u_ps0, u_ps1, cs_acc)
            for g in pending:
                g()

    nc.compile()
    return nc


def _get_compiled(n_pix, m_pix, rep=1):
    key = (n_pix, m_pix, rep)
    if key not in _COMPILED:
        _COMPILED[key] = _build_nc(n_pix, m_pix, rep)
    return _COMPILED[key]


def _in_maps(input_1, input_2, W_e, gate_w, W1, W2):
    ex = np.ascontiguousarray(input_1.reshape(B, C, HW), dtype=np.float32)
    q = np.ascontiguousarray(input_2.reshape(B, C, HW), dtype=np.float32)
    W_e = np.asarray(W_e, dtype=np.float32)
    gate_w = np.asarray(gate_w, dtype=np.float32).reshape(C, 1)
    W1 = np.asarray(W1, dtype=np.float32)
    W2 = np.asarray(W2, dtype=np.float32)

    bf = ml_dtypes.bfloat16

    def cb(a):  # contiguous bf16
        return np.ascontiguousarray(np.asarray(a).astype(bf))

    onescol = np.ones((128, 1), np.float32)
    onesrow = np.ones((1, 128), np.float32)
    gw_bf = np.ascontiguousarray(gate_w.astype(bf))
    maps = []
    for b in range(B):
        # role Q -> out2[b]
        maps.append({
            "x": ex[b], "xt": cb(ex[b].T), "y": cb(q[b]),
            "wht": np.ascontiguousarray(W_e.T),
            "wat": cb(W2[:, :C].T), "wbt": cb(W2[:, C:].T),
            "gw": gw_bf, "onescol": onescol, "onesrow": onesrow,
        })
        # role E -> out1[b]
        maps.append({
            "x": q[b], "xt": cb(q[b].T), "y": cb(ex[b]),
            "wht": np.ascontiguousarray(W_e),
            "wat": cb(W1[:, :C].T), "wbt": cb(W1[:, C:].T),
            "gw": gw_bf, "onescol": onescol, "onesrow": onesrow,
        })
    return maps


def kernel(input_1, input_2, W_e, gate_w, W1, W2):
    nc = _get_compiled(HW, HW)
    maps = _in_maps(input_1, input_2, W_e, gate_w, W1, W2)
    res = bass_utils.run_bass_kernel_spmd(
        nc, maps, core_ids=list(range(8)), trace=TRACE
    )
    kernel.last_results = res
    out1 = np.stack([res.results[2 * b + 1]["out"] for b in range(B)])
    out2 = np.stack([res.results[2 * b]["out"] for b in range(B)])
    return out1.reshape(B, C, H, W), out2.reshape(B, C, H, W)


# revision 4
# speedup vs baseline: 1.5501x; 1.1200x over previous
"""CoAttention kernel v2 for 8 TRN2 NeuronCores.

Sharding: batch (4) x role (2) = 8 cores, no collectives (see the role
symmetry note in the docstring of the original kernel).

v2 changes vs baseline:
  1. A and U matmuls use bf16 operands (PSUM accumulation stays fp32).
     fp32r self-loading matmuls pay a serialized ~107ns 4-byte
     LDWEIGHTS per instruction; bf16 weights get fast-weight-load.
     Micro-measured per-MM: f32r 332ns -> bf16 293ns.
  2. Software pipelining: the per-nj chain A->Exp->U is serialized on
     the in-order PE queue in the baseline (the U matmuls' wait for the
     ACT Exp stalls the queue, exposing ~700ns of ACT latency per nj).
     v2 issues U(nj-2) after A(nj), so Exp(nj-2) has ~2 A-pair times to
     complete before U(nj-2) reaches the head of the PE queue.
  3. The per-mj tail (colsum/gate/scale/bcast/out-conv) is a long
     cross-engine dependency chain; v2 interleaves the previous mj's
     tail groups into the next mj's inner loop at spaced slots so their
     waits are satisfied on arrival.
  EC stays fp32r (moving x / stationary Wh) for precision; ec output is
  rounded to bf16 once. Expected rel-err ~5e-3 (vs 5.5e-4 all-f32r).

Per-core program (C=256, n = X pixels, m = Y pixels):
  EC = Wh @ X                    [C, n]   (f32r, bf16 out)
  for each m-chunk (512):
    for each n-chunk (128):
      A_t  = EC_chunk^T @ Y_chunk          (PE bf16, fp32 PSUM)
      P_t  = exp(A_t - KEXP)               (ACT, PSUM->SBUF, bf16)
      cs  += P_t                           (DVE, f32 acc += bf16)
      U   += X_chunk @ P_t                 (PE bf16, fp32 PSUM)
    colsum = ones^T @ cs                   (PE f32r)  -> recip (DVE)
    gdot   = gate_w^T @ U                  (PE bf16)
    scale  = sigmoid(gdot*recip)*recip     (ACT/DVE, [1,512])
    bcast  = ones_col @ scale              (PE f32r outer product)
    out    = WaT^T @ (U*bcast) + WbT^T @ Y (PE bf16) -> DMA
"""

import numpy as np
import ml_dtypes

import concourse.bass as bass
import concourse.bacc as bacc
import concourse.tile as tile
from concourse import mybir
from concourse import bass_utils

F32 = mybir.dt.float32
F32R = mybir.dt.float32r
BF16 = mybir.dt.bfloat16

B = 4
C = 256
H = 64
W = 64
HW = H * W
KEXP = 20.0  # constant subtracted before exp (softmax-invariant)

TRACE = False
AHEAD = 2  # U(nj-AHEAD) issued after A(nj)

_COMPILED = {}


def _build_nc(n_pix, m_pix, rep=1):
    nc = bacc.Bacc(
        "TRN2",
        target_bir_lowering=False,
        debug=False,
        enable_asserts=True,
        num_devices=8,
    )
    X = nc.dram_tensor("x", [C, n_pix], F32R, kind="ExternalInput").ap()
    XT = nc.dram_tensor("xt", [n_pix, C], BF16, kind="ExternalInput").ap()
    Y = nc.dram_tensor("y", [C, m_pix], BF16, kind="ExternalInput").ap()
    WHT = nc.dram_tensor("wht", [C, C], F32R, kind="ExternalInput").ap()
    WAT = nc.dram_tensor("wat", [C, C], BF16, kind="ExternalInput").ap()
    WBT = nc.dram_tensor("wbt", [C, C], BF16, kind="ExternalInput").ap()
    GW = nc.dram_tensor("gw", [C, 1], BF16, kind="ExternalInput").ap()
    ONESC = nc.dram_tensor("onescol", [128, 1], F32R, kind="ExternalInput").ap()
    ONESR = nc.dram_tensor("onesrow", [1, 128], F32R, kind="ExternalInput").ap()
    OUT = nc.dram_tensor("out", [C, m_pix], F32, kind="ExternalOutput").ap()

    NCH = n_pix // 128
    MCH = m_pix // 512
    NK = n_pix // 512  # 512-wide n chunks for the EC phase
    Exp = mybir.ActivationFunctionType.Exp
    Copy = mybir.ActivationFunctionType.Copy

    with tile.TileContext(nc) as tc:
        with (
            nc.allow_low_precision(reason="bf16 matmul operands"),
            tc.tile_pool(name="persist", bufs=1) as persist,
            tc.tile_pool(name="psA", bufs=3, space=bass.MemorySpace.PSUM) as psA,
            tc.tile_pool(name="psU", bufs=2, space=bass.MemorySpace.PSUM) as psU,
            tc.tile_pool(name="psO", bufs=1, space=bass.MemorySpace.PSUM) as psO,
            tc.tile_pool(name="pworka", bufs=3) as pworka,
            tc.tile_pool(name="pworkb", bufs=3) as pworkb,
            tc.tile_pool(name="accp", bufs=2) as accp,
            tc.tile_pool(name="upool", bufs=2) as upool,
            tc.tile_pool(name="opool", bufs=2) as opool,
            tc.tile_pool(name="small", bufs=2) as small,
        ):
            # ---- persistent loads, ordered+chunked by first consumption ----
            Xr = X.rearrange("(ci p) n -> p ci n", p=128)
            Yr = Y.rearrange("(ci p) m -> p ci m", p=128)
            XTr = XT.rearrange("(a p) c -> p a c", p=128)
            wht_sb = persist.tile([128, 2, C], F32R)
            nc.sync.dma_start(out=wht_sb, in_=WHT.rearrange("(ci p) d -> p ci d", p=128))
            ones_col = persist.tile([128, 1], F32R)
            nc.sync.dma_start(out=ones_col, in_=ONESC)
            ones_row = persist.tile([1, 128], F32R)
            nc.sync.dma_start(out=ones_row, in_=ONESR)
            x_sb = persist.tile([128, 2, n_pix], F32R)
            for nk in range(NK):
                nsl = slice(nk * 512, (nk + 1) * 512)
                for ci in range(2):
                    nc.sync.dma_start(out=x_sb[:, ci, nsl], in_=Xr[:, ci, nsl])
            y_sb = persist.tile([128, 2, m_pix], BF16)
            for ci in range(2):
                nc.sync.dma_start(out=y_sb[:, ci, 0:512], in_=Yr[:, ci, 0:512])
            xT_sb = persist.tile([128, NCH, C], BF16)
            for a in range(0, NCH, 4):
                nc.sync.dma_start(out=xT_sb[:, a:a + 4, :], in_=XTr[:, a:a + 4, :])
            for mk in range(1, MCH):
                msl_ = slice(mk * 512, (mk + 1) * 512)
                for ci in range(2):
                    nc.sync.dma_start(out=y_sb[:, ci, msl_], in_=Yr[:, ci, msl_])
            wat_sb = persist.tile([128, 2, C], BF16)
            nc.sync.dma_start(out=wat_sb, in_=WAT.rearrange("(ci p) o -> p ci o", p=128))
            wbt_sb = persist.tile([128, 2, C], BF16)
            nc.sync.dma_start(out=wbt_sb, in_=WBT.rearrange("(ci p) o -> p ci o", p=128))
            gw_sb = persist.tile([128, 2, 1], BF16)
            nc.sync.dma_start(out=gw_sb, in_=GW.rearrange("(ci p) o -> p ci o", p=128))
            negk128 = persist.tile([128, 1], F32)
            nc.vector.memset(negk128, -KEXP)
            zero1 = persist.tile([1, 1], F32)
            nc.vector.memset(zero1, 0.0)
            ec_sb = persist.tile([128, 2, n_pix], BF16)

            # ---- EC = Wh @ X (f32r operands, bf16 result) ----
            for dj in range(2):
                for nk in range(NK):
                    nsl = slice(nk * 512, (nk + 1) * 512)
                    ec_ps = psA.tile([128, 512], F32, tag="a")
                    for ci in range(2):
                        nc.tensor.matmul(
                            ec_ps,
                            wht_sb[:, ci, dj * 128:(dj + 1) * 128],
                            x_sb[:, ci, nsl],
                            start=(ci == 0),
                            stop=(ci == 1),
                        )
                    nc.scalar.activation(ec_sb[:, dj, nsl], ec_ps, Copy)

            # ---- main loop over m-chunks (rep>1 = timing-only replay) ----
            # Tail groups of iteration t are interleaved into iteration
            # t+1's inner loop at these nj slots:
            TAIL_SLOTS = {3: 0, 6: 1, 10: 2, 13: 3, 16: 4}

            def make_tail(msl, u_ps0, u_ps1, cs_acc):
                st = {}

                def g0():  # colsum -> recip; copy U out of PSUM (bf16)
                    cs_ps = psO.tile([1, 512], F32, tag="o")
                    nc.tensor.matmul(cs_ps, ones_col, cs_acc)
                    st["recip"] = small.tile([1, 512], F32R, tag="recip", name="recip")
                    nc.vector.reciprocal(st["recip"], cs_ps)
                    st["u_sb0"] = upool.tile([128, 512], BF16, tag="usb0", name="usb0")
                    st["u_sb1"] = upool.tile([128, 512], BF16, tag="usb1", name="usb1")
                    nc.vector.tensor_copy(st["u_sb0"], u_ps0)
                    nc.vector.tensor_copy(st["u_sb1"], u_ps1)

                def g1():  # gate dot product
                    st["gd_ps"] = psO.tile([1, 512], F32, tag="o", name="gdps")
                    nc.tensor.matmul(st["gd_ps"], gw_sb[:, 0, :], st["u_sb0"],
                                     start=True, stop=False)
                    nc.tensor.matmul(st["gd_ps"], gw_sb[:, 1, :], st["u_sb1"],
                                     start=False, stop=True)

                def g2():  # scale = sigmoid(gdot/colsum)/colsum; bcast; gated
                    t_sb = small.tile([1, 512], F32R, tag="t")
                    nc.vector.tensor_mul(t_sb, st["gd_ps"], st["recip"])
                    e_sb = small.tile([1, 512], F32, tag="e")
                    nc.scalar.activation(e_sb, t_sb, Exp, bias=zero1, scale=-1.0)
                    ep1_sb = small.tile([1, 512], F32, tag="ep1")
                    nc.vector.tensor_scalar_add(ep1_sb, e_sb, 1.0)
                    g_sb = small.tile([1, 512], F32R, tag="g")
                    nc.vector.reciprocal(g_sb, ep1_sb)
                    scale_sb = small.tile([1, 512], F32R, tag="scale")
                    nc.vector.tensor_mul(scale_sb, g_sb, st["recip"])
                    bc_ps = psO.tile([128, 512], F32, tag="o")
                    nc.tensor.matmul(bc_ps, ones_row, scale_sb)
                    st["gated0"] = upool.tile([128, 512], BF16, tag="gated0", name="gated0")
                    st["gated1"] = upool.tile([128, 512], BF16, tag="gated1", name="gated1")
                    nc.vector.tensor_mul(st["gated0"], st["u_sb0"], bc_ps)
                    nc.vector.tensor_mul(st["gated1"], st["u_sb1"], bc_ps)

                def out_conv(oj):
                    osl = slice(oj * 128, (oj + 1) * 128)
                    o_ps = psO.tile([128, 512], F32, tag="o")
                    gated = [st["gated0"], st["gated1"]]
                    for ci in range(2):
                        nc.tensor.matmul(o_ps, wat_sb[:, ci, osl], gated[ci],
                                         start=(ci == 0), stop=False)
                    for ci in range(2):
                        nc.tensor.matmul(o_ps, wbt_sb[:, ci, osl], y_sb[:, ci, msl],
                                         start=False, stop=(ci == 1))
                    o_sb = opool.tile([128, 512], F32, tag="osb")
                    nc.vector.tensor_copy(o_sb, o_ps)
                    nc.sync.dma_start(out=OUT[osl, msl], in_=o_sb)

                return [g0, g1, g2, lambda: out_conv(0), lambda: out_conv(1)]

            pending = None
            for mj in [mj for _ in range(rep) for mj in range(MCH)]:
                msl = slice(mj * 512, (mj + 1) * 512)
                u_ps0 = psU.tile([128, 512], F32, tag="u0")
                u_ps1 = psU.tile([128, 512], F32, tag="u1")
                cs_acc = accp.tile([128, 512], F32R, tag="cs")
                p_tiles = {}

                def emit_U(nj):
                    nc.tensor.matmul(u_ps0, xT_sb[:, nj, 0:128], p_tiles[nj],
                                     start=(nj == 0), stop=(nj == NCH - 1))
                    nc.tensor.matmul(u_ps1, xT_sb[:, nj, 128:256], p_tiles[nj],
                                     start=(nj == 0), stop=(nj == NCH - 1))
                    del p_tiles[nj]

                for nj in range(NCH):
                    nsl128 = slice(nj * 128, (nj + 1) * 128)
                    a_ps = psA.tile([128, 512], F32, tag="a")
                    for di in range(2):
                        nc.tensor.matmul(
                            a_ps,
                            ec_sb[:, di, nsl128],
                            y_sb[:, di, msl],
                            start=(di == 0),
                            stop=(di == 1),
                        )
                    ppool = pworka if ((nj >> 1) & 1) == 0 else pworkb
                    p_sb = ppool.tile([128, 512], BF16, tag="p", name="p")
                    p_tiles[nj] = p_sb
                    nc.scalar.activation(p_sb, a_ps, Exp, bias=negk128, scale=1.0)
                    if nj == 0:
                        nc.vector.tensor_copy(cs_acc, p_sb)
                    else:
                        nc.vector.tensor_add(cs_acc, cs_acc, p_sb)
                    if nj >= AHEAD:
                        emit_U(nj - AHEAD)
                    if pending is not None and nj in TAIL_SLOTS:
                        pending[TAIL_SLOTS[nj]]()
                for nj in range(NCH - AHEAD, NCH):
                    emit_U(nj)
                pending = make_tail(msl, u_ps0, u_ps1, cs_acc)
            for g in pending:
                g()

    nc.compile()
    return nc


def _get_compiled(n_pix, m_pix, rep=1):
    key = (n_pix, m_pix, rep)
    if key not in _COMPILED:
        _COMPILED[key] = _build_nc(n_pix, m_pix, rep)
    return _COMPILED[key]


def _in_maps(input_1, input_2, W_e, gate_w, W1, W2):
    ex = np.ascontiguousarray(input_1.reshape(B, C, HW), dtype=np.float32)
    q = np.ascontiguousarray(input_2.reshape(B, C, HW), dtype=np.float32)
    W_e = np.asarray(W_e, dtype=np.float32)
    gate_w = np.asarray(gate_w, dtype=np.float32).reshape(C, 1)
    W1 = np.asarray(W1, dtype=np.float32)
    W2 = np.asarray(W2, dtype=np.float32)

    bf = ml_dtypes.bfloat16

    def cb(a):  # contiguous bf16
        return np.ascontiguousarray(np.asarray(a).astype(bf))

    onescol = np.ones((128, 1), np.float32)
    onesrow = np.ones((1, 128), np.float32)
    gw_bf = np.ascontiguousarray(gate_w.astype(bf))
    maps = []
    for b in range(B):
        # role Q -> out2[b]
        maps.append({
            "x": ex[b], "xt": cb(ex[b].T), "y": cb(q[b]),
            "wht": np.ascontiguousarray(W_e.T),
            "wat": cb(W2[:, :C].T), "wbt": cb(W2[:, C:].T),
            "gw": gw_bf, "onescol": onescol, "onesrow": onesrow,
        })
        # role E -> out1[b]
        maps.append({
            "x": q[b], "xt": cb(q[b].T), "y": cb(ex[b]),
            "wht": np.ascontiguousarray(W_e),
            "wat": cb(W1[:, :C].T), "wbt": cb(W1[:, C:].T),
            "gw": gw_bf, "onescol": onescol, "onesrow": onesrow,
        })
    return maps


def kernel(input_1, input_2, W_e, gate_w, W1, W2):
    nc = _get_compiled(HW, HW)
    maps = _in_maps(input_1, input_2, W_e, gate_w, W1, W2)
    res = bass_utils.run_bass_kernel_spmd(
        nc, maps, core_ids=list(range(8)), trace=TRACE
    )
    kernel.last_results = res
    out1 = np.stack([res.results[2 * b + 1]["out"] for b in range(B)])
    out2 = np.stack([res.results[2 * b]["out"] for b in range(B)])
    return out1.reshape(B, C, H, W), out2.reshape(B, C, H, W)


# revision 5
# speedup vs baseline: 1.5583x; 1.0053x over previous
"""CoAttention kernel v2 for 8 TRN2 NeuronCores.

Sharding: batch (4) x role (2) = 8 cores, no collectives (see the role
symmetry note in the docstring of the original kernel).

v2 changes vs baseline:
  1. A and U matmuls use bf16 operands (PSUM accumulation stays fp32).
     fp32r self-loading matmuls pay a serialized ~107ns 4-byte
     LDWEIGHTS per instruction; bf16 weights get fast-weight-load.
     Micro-measured per-MM: f32r 332ns -> bf16 293ns.
  2. Software pipelining: the per-nj chain A->Exp->U is serialized on
     the in-order PE queue in the baseline (the U matmuls' wait for the
     ACT Exp stalls the queue, exposing ~700ns of ACT latency per nj).
     v2 issues U(nj-2) after A(nj), so Exp(nj-2) has ~2 A-pair times to
     complete before U(nj-2) reaches the head of the PE queue.
  3. The per-mj tail (colsum/gate/scale/bcast/out-conv) is a long
     cross-engine dependency chain; v2 interleaves the previous mj's
     tail groups into the next mj's inner loop at spaced slots so their
     waits are satisfied on arrival.
  EC stays fp32r (moving x / stationary Wh) for precision; ec output is
  rounded to bf16 once. Expected rel-err ~5e-3 (vs 5.5e-4 all-f32r).

Per-core program (C=256, n = X pixels, m = Y pixels):
  EC = Wh @ X                    [C, n]   (f32r, bf16 out)
  for each m-chunk (512):
    for each n-chunk (128):
      A_t  = EC_chunk^T @ Y_chunk          (PE bf16, fp32 PSUM)
      P_t  = exp(A_t - KEXP)               (ACT, PSUM->SBUF, bf16)
      cs  += P_t                           (DVE, f32 acc += bf16)
      U   += X_chunk @ P_t                 (PE bf16, fp32 PSUM)
    colsum = ones^T @ cs                   (PE f32r)  -> recip (DVE)
    gdot   = gate_w^T @ U                  (PE bf16)
    scale  = sigmoid(gdot*recip)*recip     (ACT/DVE, [1,512])
    bcast  = ones_col @ scale              (PE f32r outer product)
    out    = WaT^T @ (U*bcast) + WbT^T @ Y (PE bf16) -> DMA
"""

import numpy as np
import ml_dtypes

import concourse.bass as bass
import concourse.bacc as bacc
import concourse.tile as tile
from concourse import mybir
from concourse import bass_utils

F32 = mybir.dt.float32
F32R = mybir.dt.float32r
BF16 = mybir.dt.bfloat16

B = 4
C = 256
H = 64
W = 64
HW = H * W
KEXP = 20.0  # constant subtracted before exp (softmax-invariant)

TRACE = False
AHEAD = 2  # U(nj-AHEAD) issued after A(nj)

_COMPILED = {}


def _build_nc(n_pix, m_pix, rep=1):
    nc = bacc.Bacc(
        "TRN2",
        target_bir_lowering=False,
        debug=False,
        enable_asserts=True,
        num_devices=8,
    )
    X = nc.dram_tensor("x", [C, n_pix], BF16, kind="ExternalInput").ap()
    XT = nc.dram_tensor("xt", [n_pix, C], BF16, kind="ExternalInput").ap()
    Y = nc.dram_tensor("y", [C, m_pix], BF16, kind="ExternalInput").ap()
    WHT = nc.dram_tensor("wht", [C, C], BF16, kind="ExternalInput").ap()
    WAT = nc.dram_tensor("wat", [C, C], BF16, kind="ExternalInput").ap()
    WBT = nc.dram_tensor("wbt", [C, C], BF16, kind="ExternalInput").ap()
    GW = nc.dram_tensor("gw", [C, 1], BF16, kind="ExternalInput").ap()
    ONESC = nc.dram_tensor("onescol", [128, 1], F32R, kind="ExternalInput").ap()
    ONESR = nc.dram_tensor("onesrow", [1, 128], F32R, kind="ExternalInput").ap()
    OUT = nc.dram_tensor("out", [C, m_pix], F32, kind="ExternalOutput").ap()

    NCH = n_pix // 128
    MCH = m_pix // 512
    NK = n_pix // 512  # 512-wide n chunks for the EC phase
    Exp = mybir.ActivationFunctionType.Exp
    Copy = mybir.ActivationFunctionType.Copy

    with tile.TileContext(nc) as tc:
        with (
            nc.allow_low_precision(reason="bf16 matmul operands"),
            tc.tile_pool(name="persist", bufs=1) as persist,
            tc.tile_pool(name="psA", bufs=3, space=bass.MemorySpace.PSUM) as psA,
            tc.tile_pool(name="psU", bufs=2, space=bass.MemorySpace.PSUM) as psU,
            tc.tile_pool(name="psO", bufs=1, space=bass.MemorySpace.PSUM) as psO,
            tc.tile_pool(name="pworka", bufs=3) as pworka,
            tc.tile_pool(name="pworkb", bufs=3) as pworkb,
            tc.tile_pool(name="accp", bufs=2) as accp,
            tc.tile_pool(name="upool", bufs=2) as upool,
            tc.tile_pool(name="opool", bufs=2) as opool,
            tc.tile_pool(name="small", bufs=2) as small,
        ):
            # ---- persistent loads, ordered+chunked by first consumption ----
            Xr = X.rearrange("(ci p) n -> p ci n", p=128)
            Yr = Y.rearrange("(ci p) m -> p ci m", p=128)
            XTr = XT.rearrange("(a p) c -> p a c", p=128)
            wht_sb = persist.tile([128, 2, C], BF16)
            nc.sync.dma_start(out=wht_sb, in_=WHT.rearrange("(ci p) d -> p ci d", p=128))
            ones_col = persist.tile([128, 1], F32R)
            nc.sync.dma_start(out=ones_col, in_=ONESC)
            ones_row = persist.tile([1, 128], F32R)
            nc.sync.dma_start(out=ones_row, in_=ONESR)
            x_sb = persist.tile([128, 2, n_pix], BF16)
            for nk in range(NK):
                nsl = slice(nk * 512, (nk + 1) * 512)
                for ci in range(2):
                    nc.sync.dma_start(out=x_sb[:, ci, nsl], in_=Xr[:, ci, nsl])
            y_sb = persist.tile([128, 2, m_pix], BF16)
            for ci in range(2):
                nc.sync.dma_start(out=y_sb[:, ci, 0:512], in_=Yr[:, ci, 0:512])
            xT_sb = persist.tile([128, NCH, C], BF16)
            for a in range(0, NCH, 4):
                nc.sync.dma_start(out=xT_sb[:, a:a + 4, :], in_=XTr[:, a:a + 4, :])
            for mk in range(1, MCH):
                msl_ = slice(mk * 512, (mk + 1) * 512)
                for ci in range(2):
                    nc.sync.dma_start(out=y_sb[:, ci, msl_], in_=Yr[:, ci, msl_])
            wat_sb = persist.tile([128, 2, C], BF16)
            nc.sync.dma_start(out=wat_sb, in_=WAT.rearrange("(ci p) o -> p ci o", p=128))
            wbt_sb = persist.tile([128, 2, C], BF16)
            nc.sync.dma_start(out=wbt_sb, in_=WBT.rearrange("(ci p) o -> p ci o", p=128))
            gw_sb = persist.tile([128, 2, 1], BF16)
            nc.sync.dma_start(out=gw_sb, in_=GW.rearrange("(ci p) o -> p ci o", p=128))
            negk128 = persist.tile([128, 1], F32)
            nc.vector.memset(negk128, -KEXP)
            zero1 = persist.tile([1, 1], F32)
            nc.vector.memset(zero1, 0.0)
            ec_sb = persist.tile([128, 2, n_pix], BF16)

            # ---- EC = Wh @ X (f32r operands, bf16 result) ----
            for dj in range(2):
                for nk in range(NK):
                    nsl = slice(nk * 512, (nk + 1) * 512)
                    ec_ps = psA.tile([128, 512], F32, tag="a")
                    for ci in range(2):
                        nc.tensor.matmul(
                            ec_ps,
                            wht_sb[:, ci, dj * 128:(dj + 1) * 128],
                            x_sb[:, ci, nsl],
                            start=(ci == 0),
                            stop=(ci == 1),
                        )
                    nc.scalar.activation(ec_sb[:, dj, nsl], ec_ps, Copy)

            # ---- main loop over m-chunks (rep>1 = timing-only replay) ----
            # Tail groups of iteration t are interleaved into iteration
            # t+1's inner loop at these nj slots:
            TAIL_SLOTS = {3: 0, 6: 1, 10: 2, 13: 3, 16: 4}

            def make_tail(msl, u_ps0, u_ps1, cs_acc):
                st = {}

                def g0():  # colsum -> recip; copy U out of PSUM (bf16)
                    cs_ps = psO.tile([1, 512], F32, tag="o")
                    nc.tensor.matmul(cs_ps, ones_col, cs_acc)
                    st["recip"] = small.tile([1, 512], F32R, tag="recip", name="recip")
                    nc.vector.reciprocal(st["recip"], cs_ps)
                    st["u_sb0"] = upool.tile([128, 512], BF16, tag="usb0", name="usb0")
                    st["u_sb1"] = upool.tile([128, 512], BF16, tag="usb1", name="usb1")
                    nc.vector.tensor_copy(st["u_sb0"], u_ps0)
                    nc.vector.tensor_copy(st["u_sb1"], u_ps1)

                def g1():  # gate dot product
                    st["gd_ps"] = psO.tile([1, 512], F32, tag="o", name="gdps")
                    nc.tensor.matmul(st["gd_ps"], gw_sb[:, 0, :], st["u_sb0"],
                                     start=True, stop=False)
                    nc.tensor.matmul(st["gd_ps"], gw_sb[:, 1, :], st["u_sb1"],
                                     start=False, stop=True)

                def g2():  # scale = sigmoid(gdot/colsum)/colsum; bcast; gated
                    t_sb = small.tile([1, 512], F32R, tag="t")
                    nc.vector.tensor_mul(t_sb, st["gd_ps"], st["recip"])
                    e_sb = small.tile([1, 512], F32, tag="e")
                    nc.scalar.activation(e_sb, t_sb, Exp, bias=zero1, scale=-1.0)
                    ep1_sb = small.tile([1, 512], F32, tag="ep1")
                    nc.vector.tensor_scalar_add(ep1_sb, e_sb, 1.0)
                    g_sb = small.tile([1, 512], F32R, tag="g")
                    nc.vector.reciprocal(g_sb, ep1_sb)
                    scale_sb = small.tile([1, 512], F32R, tag="scale")
                    nc.vector.tensor_mul(scale_sb, g_sb, st["recip"])
                    bc_ps = psO.tile([128, 512], F32, tag="o")
                    nc.tensor.matmul(bc_ps, ones_row, scale_sb)
                    st["gated0"] = upool.tile([128, 512], BF16, tag="gated0", name="gated0")
                    st["gated1"] = upool.tile([128, 512], BF16, tag="gated1", name="gated1")
                    nc.vector.tensor_mul(st["gated0"], st["u_sb0"], bc_ps)
                    nc.vector.tensor_mul(st["gated1"], st["u_sb1"], bc_ps)

                def out_conv(oj):
                    osl = slice(oj * 128, (oj + 1) * 128)
                    o_ps = psO.tile([128, 512], F32, tag="o")
                    gated = [st["gated0"], st["gated1"]]
                    for ci in range(2):
                        nc.tensor.matmul(o_ps, wat_sb[:, ci, osl], gated[ci],
                                         start=(ci == 0), stop=False)
                    for ci in range(2):
                        nc.tensor.matmul(o_ps, wbt_sb[:, ci, osl], y_sb[:, ci, msl],
                                         start=False, stop=(ci == 1))
                    o_sb = opool.tile([128, 512], F32, tag="osb")
                    nc.vector.tensor_copy(o_sb, o_ps)
                    nc.sync.dma_start(out=OUT[osl, msl], in_=o_sb)

                return [g0, g1, g2, lambda: out_conv(0), lambda: out_conv(1)]

            pending = None
            for mj in [mj for _ in range(rep) for mj in range(MCH)]:
                msl = slice(mj * 512, (mj + 1) * 512)
                u_ps0 = psU.tile([128, 512], F32, tag="u0")
                u_ps1 = psU.tile([128, 512], F32, tag="u1")
                cs_acc = accp.tile([128, 512], F32R, tag="cs")
                p_tiles = {}

                def emit_U(nj):
                    nc.tensor.matmul(u_ps0, xT_sb[:, nj, 0:128], p_tiles[nj],
                                     start=(nj == 0), stop=(nj == NCH - 1))
                    nc.tensor.matmul(u_ps1, xT_sb[:, nj, 128:256], p_tiles[nj],
                                     start=(nj == 0), stop=(nj == NCH - 1))
                    del p_tiles[nj]

                for nj in range(NCH):
                    nsl128 = slice(nj * 128, (nj + 1) * 128)
                    a_ps = psA.tile([128, 512], F32, tag="a")
                    for di in range(2):
                        nc.tensor.matmul(
                            a_ps,
                            ec_sb[:, di, nsl128],
                            y_sb[:, di, msl],
                            start=(di == 0),
                            stop=(di == 1),
                        )
                    ppool = pworka if ((nj >> 1) & 1) == 0 else pworkb
                    p_sb = ppool.tile([128, 512], BF16, tag="p", name="p")
                    p_tiles[nj] = p_sb
                    nc.scalar.activation(p_sb, a_ps, Exp, bias=negk128, scale=1.0)
                    if nj == 0:
                        nc.vector.tensor_copy(cs_acc, p_sb)
                    else:
                        nc.vector.tensor_add(cs_acc, cs_acc, p_sb)
                    if nj >= AHEAD:
                        emit_U(nj - AHEAD)
                    if pending is not None and nj in TAIL_SLOTS:
                        pending[TAIL_SLOTS[nj]]()
                for nj in range(NCH - AHEAD, NCH):
                    emit_U(nj)
                pending = make_tail(msl, u_ps0, u_ps1, cs_acc)
            for g in pending:
                g()

    nc.compile()
    return nc


def _get_compiled(n_pix, m_pix, rep=1):
    key = (n_pix, m_pix, rep)
    if key not in _COMPILED:
        _COMPILED[key] = _build_nc(n_pix, m_pix, rep)
    return _COMPILED[key]


def _in_maps(input_1, input_2, W_e, gate_w, W1, W2):
    ex = np.ascontiguousarray(input_1.reshape(B, C, HW), dtype=np.float32)
    q = np.ascontiguousarray(input_2.reshape(B, C, HW), dtype=np.float32)
    W_e = np.asarray(W_e, dtype=np.float32)
    gate_w = np.asarray(gate_w, dtype=np.float32).reshape(C, 1)
    W1 = np.asarray(W1, dtype=np.float32)
    W2 = np.asarray(W2, dtype=np.float32)

    bf = ml_dtypes.bfloat16

    def cb(a):  # contiguous bf16
        return np.ascontiguousarray(np.asarray(a).astype(bf))

    onescol = np.ones((128, 1), np.float32)
    onesrow = np.ones((1, 128), np.float32)
    gw_bf = np.ascontiguousarray(gate_w.astype(bf))
    maps = []
    for b in range(B):
        # role Q -> out2[b]
        maps.append({
            "x": cb(ex[b]), "xt": cb(ex[b].T), "y": cb(q[b]),
            "wht": cb(W_e.T),
            "wat": cb(W2[:, :C].T), "wbt": cb(W2[:, C:].T),
            "gw": gw_bf, "onescol": onescol, "onesrow": onesrow,
        })
        # role E -> out1[b]
        maps.append({
            "x": cb(q[b]), "xt": cb(q[b].T), "y": cb(ex[b]),
            "wht": cb(W_e),
            "wat": cb(W1[:, :C].T), "wbt": cb(W1[:, C:].T),
            "gw": gw_bf, "onescol": onescol, "onesrow": onesrow,
        })
    return maps


def kernel(input_1, input_2, W_e, gate_w, W1, W2):
    nc = _get_compiled(HW, HW)
    maps = _in_maps(input_1, input_2, W_e, gate_w, W1, W2)
    res = bass_utils.run_bass_kernel_spmd(
        nc, maps, core_ids=list(range(8)), trace=TRACE
    )
    kernel.last_results = res
    out1 = np.stack([res.results[2 * b + 1]["out"] for b in range(B)])
    out2 = np.stack([res.results[2 * b]["out"] for b in range(B)])
    return out1.reshape(B, C, H, W), out2.reshape(B, C, H, W)
